# revision 4
# baseline (speedup 1.0000x reference)
"""LOGG3D_ATTN self-attention kernel for Trainium2 — For_i-loop edition.

Math (identical to the baseline kernel):
    raw[i, j] = sum_d feats[i, d] * feats[j, d]            (PE, f32r)
    E[j, i]   = exp(raw[i, j] / 4)                         (ScalarE, PSUM->SBUF)
    ctx_aug   = sum_j E[j, i] * [feats[j, :], 1]           (PE, accumulating)
exp is computed without the row-max subtraction (scores are well inside fp32
exp range), softmax normalization happens on the host via the appended ones
column.

Structure: the per-core program is a nested hardware loop
    For_i(outer: T repeats)            # T=1 in production, >1 for timing
      For_i(ic: 3 i-chunks of 512)     # dynamic slice ts(ic, 512) of shardT
        Phase A: 32 groups x (3 score MMs -> exp into staged E_sb, bf16)
        Phase B: 96 ctx MMs as one uninterrupted PSUM accumulation run
      copy chunk -> ctx_sb; DMA out per outer iteration
so the program stays at ~250 instructions regardless of T.  This matters
because on this backend each *emitted* instruction costs ~65 us of
per-execution overhead (measured: an unrolled body costs ~8.4 ms/iteration
regardless of the work in it, while the same body in a For_i loop costs
~90 us/iteration) — hardware loops keep the program tiny.

Phases A and B are split (rather than interleaved) because mixing the
K=16 score matmuls with the K=128 ctx matmuls on the PE measurably slows
both; staging E in bf16 makes the full [128, 96*512] chunk fit in SBUF.

Each core gets the full featsT plus its own shardT (its 1536 attention
rows).  Outputs are the per-core ctx_aug [17, 1536].
"""

import math
import time

import numpy as np

import concourse.bacc as bacc
import concourse.bass as bass
import concourse.mybir as mybir
import concourse.tile as tile
from concourse import bass_utils

N_POINTS = 12288
FEAT_DIM = 16
N_CORES = 8

IC = 512          # i-chunk width (PSUM bank)
JG = 3            # j-tiles per exp group
JT = N_POINTS // 128
NIC_PER_CORE = 3  # 1536 / 512

last_profile = {}
_program_cache = {}


def build_loop_program(T=1, N=N_POINTS, D=FEAT_DIM):
    """Per-core SPMD program with nested hardware loops. T = outer repeats."""
    key = ("loop", T, N, D)
    if key in _program_cache:
        return _program_cache[key]

    R = N // N_CORES
    n_groups = JT // JG
    assert JT % JG == 0 and R == NIC_PER_CORE * IC

    f32 = mybir.dt.float32
    f32r = mybir.dt.float32r
    EXP = mybir.ActivationFunctionType.Exp

    nc = bacc.Bacc("TRN2", target_bir_lowering=False, debug=False)

    featsT_d = nc.dram_tensor("featsT", [D, N], f32r, kind="ExternalInput")
    shardT_d = nc.dram_tensor("shardT", [D, R], f32r, kind="ExternalInput")
    bf16 = mybir.dt.bfloat16
    aug_d = nc.dram_tensor("aug", [128, JT, D + 1], bf16, kind="ExternalInput")
    out_d = nc.dram_tensor("ctx_out", [D + 1, R], f32, kind="ExternalOutput")

    with tile.TileContext(nc) as tc:
        with (
            tc.tile_pool(name="const", bufs=1) as cpool,
            tc.tile_pool(name="st", bufs=1, space="PSUM") as st_pool,
            tc.tile_pool(name="ctxp", bufs=1, space="PSUM") as ctx_pool,
            tc.tile_pool(name="e", bufs=1) as e_pool,
            tc.tile_pool(name="out", bufs=1) as out_pool,
        ):
            featsT_sb = cpool.tile([D, N], f32r)
            nc.sync.dma_start(featsT_sb[:], featsT_d[:])
            shardT_sb = cpool.tile([D, R], f32r)
            nc.sync.dma_start(shardT_sb[:], shardT_d[:])
            aug_sb = cpool.tile([128, JT, D + 1], bf16)
            nc.sync.dma_start(aug_sb[:], aug_d[:])

            # static buffers, rotated by python index -> no pool/loop magic
            st_tiles = [st_pool.tile([128, JG * IC], f32, tag=f"st{b}", name=f"st{b}")
                        for b in range(2)]
            E_sb = e_pool.tile([128, JT * IC], bf16, tag="E", name="E_sb")
            ctx_ps = ctx_pool.tile([D + 1, IC], f32, tag="ctx", name="ctx_ps")
            ctx_sb = out_pool.tile([D + 1, R], f32, tag="out", name="ctx_sb")

            with tc.For_i(0, T, 1, name="rep", hint_engines=(mybir.EngineType.PE,)):
                with tc.For_i(0, NIC_PER_CORE, 1, name="chunk", hint_engines=(mybir.EngineType.PE,)) as ic:
                    rhs1 = shardT_sb[:, bass.ts(ic, IC)]
                    # Phase A: all score MMs + exp, staged into E_sb (bf16).
                    for g in range(n_groups):
                        st = st_tiles[g % 2]
                        for jj in range(JG):
                            j = g * JG + jj
                            nc.tensor.matmul(
                                st[:, jj * IC:(jj + 1) * IC],
                                featsT_sb[:, j * 128:(j + 1) * 128],
                                rhs1,
                                start=True,
                                stop=True,
                            )
                        nc.scalar.activation(
                            E_sb[:, g * JG * IC:(g + 1) * JG * IC], st[:],
                            EXP, scale=0.25)
                    # Phase B: one uninterrupted ctx accumulation run.
                    for j in range(JT):
                        nc.tensor.matmul(
                            ctx_ps[:, :],
                            aug_sb[:, j, :],
                            E_sb[:, j * IC:(j + 1) * IC],
                            start=(j == 0),
                            stop=(j == JT - 1),
                        )
                    nc.vector.tensor_copy(ctx_sb[:, bass.ts(ic, IC)], ctx_ps[:])
                nc.sync.dma_start(out_d[:], ctx_sb[:])

    nc.compile()
    _program_cache[key] = nc
    return nc


def make_in_maps(feats, N=N_POINTS, D=FEAT_DIM):
    featsT = np.ascontiguousarray(feats.T).astype(np.float32)            # [D, N]
    aug = np.concatenate(
        [feats.astype(np.float32), np.ones((N, 1), np.float32)], axis=1
    )                                                                     # [N, D+1]
    import ml_dtypes
    aug_tiled = np.ascontiguousarray(
        aug.reshape(JT, 128, D + 1).transpose(1, 0, 2)
    ).astype(ml_dtypes.bfloat16)                                          # [128, JT, D+1]
    R = N // N_CORES
    in_maps = []
    for c in range(N_CORES):
        shardT = np.ascontiguousarray(featsT[:, c * R:(c + 1) * R])
        in_maps.append({"featsT": featsT, "shardT": shardT, "aug": aug_tiled})
    return in_maps


def run_program(nc, in_maps):
    res = None
    for attempt in range(3):
        try:
            res = bass_utils.run_bass_kernel_spmd(nc, in_maps, list(range(N_CORES)))
            break
        except Exception:
            if attempt == 2:
                raise
            time.sleep(5.0 * (attempt + 1))
    global last_profile
    last_profile = {
        "exec_time_ns": res.exec_time_ns,
        "mean_exec_time_ns": res.mean_exec_time_ns,
    }
    return res


def attention_ctx_on_device(feats, T=1):
    nc = build_loop_program(T=T)
    in_maps = make_in_maps(feats)
    res = run_program(nc, in_maps)
    ctx = np.concatenate(
        [res.results[c]["ctx_out"] for c in range(N_CORES)], axis=1
    )                                                                     # [D+1, N]
    return ctx


def _epilogue(feats, topK, ctx_aug, N, D):
    num = np.einsum("dn,nd->n", ctx_aug[:D].astype(np.float64), feats.astype(np.float64))
    Z = ctx_aug[D].astype(np.float64)
    w = 1.0 / (1.0 + np.exp(-(num / Z)))                                  # sigmoid, [N]

    weighted = feats * w[:, None].astype(np.float32)                      # [N, D]
    k = int(N * np.asarray(topK).item())
    if k >= N:
        sel = weighted
    else:
        top_idx = np.argsort(-w, kind="stable")[:k]
        sel = weighted[top_idx]
    so = (sel.T.astype(np.float32) @ sel.astype(np.float32)) / np.float32(max(k, 1))
    out = so.reshape(1, -1).astype(np.float32)
    nrm = np.linalg.norm(out, axis=-1, keepdims=True).astype(np.float32)
    return (out / nrm).astype(np.float32)


def kernel(feats, topK):
    feats = np.asarray(feats, dtype=np.float32)
    N, D = feats.shape
    ctx_aug = attention_ctx_on_device(feats, T=1)
    return _epilogue(feats, topK, ctx_aug, N, D)


# revision 5
# speedup vs baseline: 1.0010x; 1.0010x over previous
"""LOGG3D_ATTN self-attention kernel for Trainium2 — For_i-loop edition.

Math (identical to the baseline kernel):
    raw[i, j] = sum_d feats[i, d] * feats[j, d]            (PE, f32r)
    E[j, i]   = exp(raw[i, j] / 4)                         (ScalarE, PSUM->SBUF)
    ctx_aug   = sum_j E[j, i] * [feats[j, :], 1]           (PE, accumulating)
exp is computed without the row-max subtraction (scores are well inside fp32
exp range), softmax normalization happens on the host via the appended ones
column.

Structure: the per-core program is a nested hardware loop
    For_i(outer: T repeats)            # T=1 in production, >1 for timing
      For_i(ic: 3 i-chunks of 512)     # dynamic slice ts(ic, 512) of shardT
        Phase A: 32 groups x (3 score MMs -> exp into staged E_sb, bf16)
        Phase B: 96 ctx MMs as one uninterrupted PSUM accumulation run
      copy chunk -> ctx_sb; DMA out per outer iteration
so the program stays at ~250 instructions regardless of T.  This matters
because on this backend each *emitted* instruction costs ~65 us of
per-execution overhead (measured: an unrolled body costs ~8.4 ms/iteration
regardless of the work in it, while the same body in a For_i loop costs
~90 us/iteration) — hardware loops keep the program tiny.

Phases A and B are split (rather than interleaved) because mixing the
K=16 score matmuls with the K=128 ctx matmuls on the PE measurably slows
both; staging E in bf16 makes the full [128, 96*512] chunk fit in SBUF.

Each core gets the full featsT plus its own shardT (its 1536 attention
rows).  Outputs are the per-core ctx_aug [17, 1536].
"""

import math
import time

import numpy as np

import concourse.bacc as bacc
import concourse.bass as bass
import concourse.mybir as mybir
import concourse.tile as tile
from concourse import bass_utils

N_POINTS = 12288
FEAT_DIM = 16
N_CORES = 8

IC = 512          # i-chunk width (PSUM bank)
JG = 3            # j-tiles per exp group
JT = N_POINTS // 128
NIC_PER_CORE = 3  # 1536 / 512

last_profile = {}
_program_cache = {}


def build_loop_program(T=1, N=N_POINTS, D=FEAT_DIM):
    """Per-core SPMD program with nested hardware loops. T = outer repeats."""
    key = ("loop", T, N, D)
    if key in _program_cache:
        return _program_cache[key]

    R = N // N_CORES
    n_groups = JT // JG
    assert JT % JG == 0 and R == NIC_PER_CORE * IC

    f32 = mybir.dt.float32
    f32r = mybir.dt.float32r
    EXP = mybir.ActivationFunctionType.Exp

    nc = bacc.Bacc("TRN2", target_bir_lowering=False, debug=False)

    featsT_d = nc.dram_tensor("featsT", [D, N], f32r, kind="ExternalInput")
    shardT_d = nc.dram_tensor("shardT", [D, R], f32r, kind="ExternalInput")
    bf16 = mybir.dt.bfloat16
    aug_d = nc.dram_tensor("aug", [128, JT, D + 1], bf16, kind="ExternalInput")
    out_d = nc.dram_tensor("ctx_out", [D + 1, R], f32, kind="ExternalOutput")

    with tile.TileContext(nc) as tc:
        with (
            tc.tile_pool(name="const", bufs=1) as cpool,
            tc.tile_pool(name="st", bufs=1, space="PSUM") as st_pool,
            tc.tile_pool(name="ctxp", bufs=1, space="PSUM") as ctx_pool,
            tc.tile_pool(name="e", bufs=1) as e_pool,
            tc.tile_pool(name="out", bufs=1) as out_pool,
        ):
            featsT_sb = cpool.tile([D, N], f32r)
            nc.sync.dma_start(featsT_sb[:], featsT_d[:])
            shardT_sb = cpool.tile([D, R], f32r)
            nc.sync.dma_start(shardT_sb[:], shardT_d[:])
            aug_sb = cpool.tile([128, JT, D + 1], bf16)
            nc.sync.dma_start(aug_sb[:], aug_d[:])

            # static buffers, rotated by python index -> no pool/loop magic
            st_tiles = [st_pool.tile([128, JG * IC], f32, tag=f"st{b}", name=f"st{b}")
                        for b in range(2)]
            E_sb = e_pool.tile([128, JT * IC], bf16, tag="E", name="E_sb")
            ctx_ps = ctx_pool.tile([D + 1, IC], f32, tag="ctx", name="ctx_ps")
            ctx_sb = out_pool.tile([D + 1, R], f32, tag="out", name="ctx_sb")

            with tc.For_i(0, T, 1, name="rep", hint_engines=(mybir.EngineType.PE,), staggered_reset=True):
                with tc.For_i(0, NIC_PER_CORE, 1, name="chunk", hint_engines=(mybir.EngineType.PE,), staggered_reset=True) as ic:
                    rhs1 = shardT_sb[:, bass.ts(ic, IC)]
                    # Phase A: all score MMs + exp, staged into E_sb (bf16).
                    for g in range(n_groups):
                        st = st_tiles[g % 2]
                        for jj in range(JG):
                            j = g * JG + jj
                            nc.tensor.matmul(
                                st[:, jj * IC:(jj + 1) * IC],
                                featsT_sb[:, j * 128:(j + 1) * 128],
                                rhs1,
                                start=True,
                                stop=True,
                            )
                        nc.scalar.activation(
                            E_sb[:, g * JG * IC:(g + 1) * JG * IC], st[:],
                            EXP, scale=0.25)
                    # Phase B: one uninterrupted ctx accumulation run.
                    for j in range(JT):
                        nc.tensor.matmul(
                            ctx_ps[:, :],
                            aug_sb[:, j, :],
                            E_sb[:, j * IC:(j + 1) * IC],
                            start=(j == 0),
                            stop=(j == JT - 1),
                        )
                    nc.vector.tensor_copy(ctx_sb[:, bass.ts(ic, IC)], ctx_ps[:])
                nc.sync.dma_start(out_d[:], ctx_sb[:])

    nc.compile()
    _program_cache[key] = nc
    return nc


def make_in_maps(feats, N=N_POINTS, D=FEAT_DIM):
    featsT = np.ascontiguousarray(feats.T).astype(np.float32)            # [D, N]
    aug = np.concatenate(
        [feats.astype(np.float32), np.ones((N, 1), np.float32)], axis=1
    )                                                                     # [N, D+1]
    import ml_dtypes
    aug_tiled = np.ascontiguousarray(
        aug.reshape(JT, 128, D + 1).transpose(1, 0, 2)
    ).astype(ml_dtypes.bfloat16)                                          # [128, JT, D+1]
    R = N // N_CORES
    in_maps = []
    for c in range(N_CORES):
        shardT = np.ascontiguousarray(featsT[:, c * R:(c + 1) * R])
        in_maps.append({"featsT": featsT, "shardT": shardT, "aug": aug_tiled})
    return in_maps


def run_program(nc, in_maps):
    res = None
    for attempt in range(3):
        try:
            res = bass_utils.run_bass_kernel_spmd(nc, in_maps, list(range(N_CORES)))
            break
        except Exception:
            if attempt == 2:
                raise
            time.sleep(5.0 * (attempt + 1))
    global last_profile
    last_profile = {
        "exec_time_ns": res.exec_time_ns,
        "mean_exec_time_ns": res.mean_exec_time_ns,
    }
    return res


def attention_ctx_on_device(feats, T=1):
    nc = build_loop_program(T=T)
    in_maps = make_in_maps(feats)
    res = run_program(nc, in_maps)
    ctx = np.concatenate(
        [res.results[c]["ctx_out"] for c in range(N_CORES)], axis=1
    )                                                                     # [D+1, N]
    return ctx


def _epilogue(feats, topK, ctx_aug, N, D):
    num = np.einsum("dn,nd->n", ctx_aug[:D].astype(np.float64), feats.astype(np.float64))
    Z = ctx_aug[D].astype(np.float64)
    w = 1.0 / (1.0 + np.exp(-(num / Z)))                                  # sigmoid, [N]

    weighted = feats * w[:, None].astype(np.float32)                      # [N, D]
    k = int(N * np.asarray(topK).item())
    if k >= N:
        sel = weighted
    else:
        top_idx = np.argsort(-w, kind="stable")[:k]
        sel = weighted[top_idx]
    so = (sel.T.astype(np.float32) @ sel.astype(np.float32)) / np.float32(max(k, 1))
    out = so.reshape(1, -1).astype(np.float32)
    nrm = np.linalg.norm(out, axis=-1, keepdims=True).astype(np.float32)
    return (out / nrm).astype(np.float32)


def kernel(feats, topK):
    feats = np.asarray(feats, dtype=np.float32)
    N, D = feats.shape
    ctx_aug = attention_ctx_on_device(feats, T=1)
    return _epilogue(feats, topK, ctx_aug, N, D)


# revision 6
# speedup vs baseline: 1.7411x; 1.7393x over previous
"""LOGG3D_ATTN self-attention kernel for Trainium2 — For_i-loop edition.

Math (identical to the baseline kernel):
    raw[i, j] = sum_d feats[i, d] * feats[j, d]            (PE, f32r)
    E[j, i]   = exp(raw[i, j] / 4)                         (ScalarE, PSUM->SBUF)
    ctx_aug   = sum_j E[j, i] * [feats[j, :], 1]           (PE, accumulating)
exp is computed without the row-max subtraction (scores are well inside fp32
exp range), softmax normalization happens on the host via the appended ones
column.

Structure: the per-core program is a nested hardware loop
    For_i(outer: T repeats)            # T=1 in production, >1 for timing
      For_i(ic: 3 i-chunks of 512)     # dynamic slice ts(ic, 512) of shardT
        Phase A: 32 groups x (3 score MMs -> exp into staged E_sb, bf16)
        Phase B: 96 ctx MMs as one uninterrupted PSUM accumulation run
      copy chunk -> ctx_sb; DMA out per outer iteration
so the program stays at ~250 instructions regardless of T.  This matters
because on this backend each *emitted* instruction costs ~65 us of
per-execution overhead (measured: an unrolled body costs ~8.4 ms/iteration
regardless of the work in it, while the same body in a For_i loop costs
~90 us/iteration) — hardware loops keep the program tiny.

Phases A and B are split (rather than interleaved) because mixing the
K=16 score matmuls with the K=128 ctx matmuls on the PE measurably slows
both; staging E in bf16 makes the full [128, 96*512] chunk fit in SBUF.

Each core gets the full featsT plus its own shardT (its 1536 attention
rows).  Outputs are the per-core ctx_aug [17, 1536].
"""

import math
import time

import numpy as np

import concourse.bacc as bacc
import concourse.bass as bass
import concourse.mybir as mybir
import concourse.tile as tile
from concourse import bass_utils

N_POINTS = 12288
FEAT_DIM = 16
N_CORES = 8

IC = 512          # i-chunk width (PSUM bank)
JG = 3            # j-tiles per exp group
JT = N_POINTS // 128
NIC_PER_CORE = 3  # 1536 / 512

last_profile = {}
_program_cache = {}


def build_loop_program(T=1, N=N_POINTS, D=FEAT_DIM):
    """Per-core SPMD program with nested hardware loops. T = outer repeats."""
    key = ("loop", T, N, D)
    if key in _program_cache:
        return _program_cache[key]

    R = N // N_CORES
    n_groups = JT // JG
    assert JT % JG == 0 and R == NIC_PER_CORE * IC

    f32 = mybir.dt.float32
    f32r = mybir.dt.float32r
    EXP = mybir.ActivationFunctionType.Exp

    nc = bacc.Bacc("TRN2", target_bir_lowering=False, debug=False)

    featsT_d = nc.dram_tensor("featsT", [D, N], f32r, kind="ExternalInput")
    shardT_d = nc.dram_tensor("shardT", [D, R], f32r, kind="ExternalInput")
    bf16 = mybir.dt.bfloat16
    aug_d = nc.dram_tensor("aug", [128, JT, D + 1], bf16, kind="ExternalInput")
    out_d = nc.dram_tensor("ctx_out", [D + 1, R], f32, kind="ExternalOutput")

    with tile.TileContext(nc) as tc:
        with (
            tc.tile_pool(name="const", bufs=1) as cpool,
            tc.tile_pool(name="st", bufs=1, space="PSUM") as st_pool,
            tc.tile_pool(name="ctxp", bufs=1, space="PSUM") as ctx_pool,
            tc.tile_pool(name="e", bufs=1) as e_pool,
            tc.tile_pool(name="out", bufs=1) as out_pool,
        ):
            # featsT/shardT replicated into partition groups 0/32/64 so the
            # three matmuls of a JG-group run in distinct 32-row PE tiles
            # (tile_position row tiling) concurrently.
            feats4 = cpool.tile([128, N], f32r, name="feats4")
            shard4 = cpool.tile([128, R], f32r, name="shard4")
            for r in range(JG):
                nc.sync.dma_start(feats4[32 * r:32 * r + D, :], featsT_d[:])
                nc.sync.dma_start(shard4[32 * r:32 * r + D, :], shardT_d[:])
            aug_sb = cpool.tile([128, JT, D + 1], bf16)
            nc.sync.dma_start(aug_sb[:], aug_d[:])

            # static buffers, rotated by python index -> no pool/loop magic
            st_tiles = [st_pool.tile([128, JG * IC], f32, tag=f"st{b}", name=f"st{b}")
                        for b in range(2)]
            # fixed staging tile for the current chunk's rhs: tile_position
            # matmuls reject register-offset APs, so the dynamic chunk slice
            # is materialized here by a DVE copy (which handles them fine)
            rhs_stage = cpool.tile([128, IC], f32r, name="rhs_stage")
            E_sb = e_pool.tile([128, JT * IC], bf16, tag="E", name="E_sb")
            ctx_ps = ctx_pool.tile([D + 1, IC], f32, tag="ctx", name="ctx_ps")
            ctx_sb = out_pool.tile([D + 1, R], f32, tag="out", name="ctx_sb")

            with tc.For_i(0, T, 1, name="rep", hint_engines=(mybir.EngineType.PE,), staggered_reset=True):
                with tc.For_i(0, NIC_PER_CORE, 1, name="chunk", hint_engines=(mybir.EngineType.PE,), staggered_reset=True) as ic:
                    nc.vector.tensor_copy(rhs_stage[:], shard4[:, bass.ts(ic, IC)])
                    # Phase A: all score MMs + exp, staged into E_sb (bf16).
                    # Each jj targets a distinct PSUM bank and a distinct
                    # 32-row PE tile -> 3 concurrent score matmuls.
                    for g in range(n_groups):
                        st = st_tiles[g % 2]
                        for jj in range(JG):
                            j = g * JG + jj
                            nc.tensor.matmul(
                                st[:, jj * IC:(jj + 1) * IC],
                                feats4[32 * jj:32 * jj + D, j * 128:(j + 1) * 128],
                                rhs_stage[32 * jj:32 * jj + D, :],
                                start=True,
                                stop=True,
                                tile_position=(32 * jj, 0),
                            )
                        nc.scalar.activation(
                            E_sb[:, g * JG * IC:(g + 1) * JG * IC], st[:],
                            EXP, scale=0.25)
                    # Phase B: one uninterrupted ctx accumulation run.
                    for j in range(JT):
                        nc.tensor.matmul(
                            ctx_ps[:, :],
                            aug_sb[:, j, :],
                            E_sb[:, j * IC:(j + 1) * IC],
                            start=(j == 0),
                            stop=(j == JT - 1),
                        )
                    nc.vector.tensor_copy(ctx_sb[:, bass.ts(ic, IC)], ctx_ps[:])
                nc.sync.dma_start(out_d[:], ctx_sb[:])

    nc.compile()
    _program_cache[key] = nc
    return nc


def make_in_maps(feats, N=N_POINTS, D=FEAT_DIM):
    featsT = np.ascontiguousarray(feats.T).astype(np.float32)            # [D, N]
    aug = np.concatenate(
        [feats.astype(np.float32), np.ones((N, 1), np.float32)], axis=1
    )                                                                     # [N, D+1]
    import ml_dtypes
    aug_tiled = np.ascontiguousarray(
        aug.reshape(JT, 128, D + 1).transpose(1, 0, 2)
    ).astype(ml_dtypes.bfloat16)                                          # [128, JT, D+1]
    R = N // N_CORES
    in_maps = []
    for c in range(N_CORES):
        shardT = np.ascontiguousarray(featsT[:, c * R:(c + 1) * R])
        in_maps.append({"featsT": featsT, "shardT": shardT, "aug": aug_tiled})
    return in_maps


def run_program(nc, in_maps):
    res = None
    for attempt in range(3):
        try:
            res = bass_utils.run_bass_kernel_spmd(nc, in_maps, list(range(N_CORES)))
            break
        except Exception:
            if attempt == 2:
                raise
            time.sleep(5.0 * (attempt + 1))
    global last_profile
    last_profile = {
        "exec_time_ns": res.exec_time_ns,
        "mean_exec_time_ns": res.mean_exec_time_ns,
    }
    return res


def attention_ctx_on_device(feats, T=1):
    nc = build_loop_program(T=T)
    in_maps = make_in_maps(feats)
    res = run_program(nc, in_maps)
    ctx = np.concatenate(
        [res.results[c]["ctx_out"] for c in range(N_CORES)], axis=1
    )                                                                     # [D+1, N]
    return ctx


def _epilogue(feats, topK, ctx_aug, N, D):
    num = np.einsum("dn,nd->n", ctx_aug[:D].astype(np.float64), feats.astype(np.float64))
    Z = ctx_aug[D].astype(np.float64)
    w = 1.0 / (1.0 + np.exp(-(num / Z)))                                  # sigmoid, [N]

    weighted = feats * w[:, None].astype(np.float32)                      # [N, D]
    k = int(N * np.asarray(topK).item())
    if k >= N:
        sel = weighted
    else:
        top_idx = np.argsort(-w, kind="stable")[:k]
        sel = weighted[top_idx]
    so = (sel.T.astype(np.float32) @ sel.astype(np.float32)) / np.float32(max(k, 1))
    out = so.reshape(1, -1).astype(np.float32)
    nrm = np.linalg.norm(out, axis=-1, keepdims=True).astype(np.float32)
    return (out / nrm).astype(np.float32)


def kernel(feats, topK):
    feats = np.asarray(feats, dtype=np.float32)
    N, D = feats.shape
    ctx_aug = attention_ctx_on_device(feats, T=1)
    return _epilogue(feats, topK, ctx_aug, N, D)


# revision 7
# speedup vs baseline: 1.8220x; 1.0465x over previous
"""LOGG3D_ATTN self-attention kernel for Trainium2 — For_i-loop edition.

Math (identical to the baseline kernel):
    raw[i, j] = sum_d feats[i, d] * feats[j, d]            (PE, f32r)
    E[j, i]   = exp(raw[i, j] / 4)                         (ScalarE, PSUM->SBUF)
    ctx_aug   = sum_j E[j, i] * [feats[j, :], 1]           (PE, accumulating)
exp is computed without the row-max subtraction (scores are well inside fp32
exp range), softmax normalization happens on the host via the appended ones
column.

Structure: the per-core program is a nested hardware loop
    For_i(outer: T repeats)            # T=1 in production, >1 for timing
      For_i(ic: 3 i-chunks of 512)     # chunk rhs staged by one DVE copy
        Phase A: 32 groups x (3 row-tiled score MMs -> exp into E_sb, bf16)
        Phase B: 96 ctx MMs as one uninterrupted PSUM accumulation run
      copy chunk -> ctx_sb; DMA out per outer iteration

The K=16 score matmuls use tile_position row tiling: featsT/shardT are
replicated into SBUF partition groups 0/32/64, and the three matmuls of a
group run concurrently in distinct 32-row PE tiles writing distinct PSUM
banks (~4x score-matmul throughput, bit-exact).  tile_position matmuls
reject register-offset APs, so the chunk's moving operand is materialized
into a fixed staging tile by a DVE copy first.
so the program stays at ~250 instructions regardless of T.  This matters
because on this backend each *emitted* instruction costs ~65 us of
per-execution overhead (measured: an unrolled body costs ~8.4 ms/iteration
regardless of the work in it, while the same body in a For_i loop costs
~90 us/iteration) — hardware loops keep the program tiny.

Phases A and B are split (rather than interleaved) because mixing the
K=16 score matmuls with the K=128 ctx matmuls on the PE measurably slows
both; staging E in bf16 makes the full [128, 96*512] chunk fit in SBUF.

Each core gets the full featsT plus its own shardT (its 1536 attention
rows).  Outputs are the per-core ctx_aug [17, 1536].
"""

import math
import time

import numpy as np

import concourse.bacc as bacc
import concourse.bass as bass
import concourse.mybir as mybir
import concourse.tile as tile
from concourse import bass_utils

N_POINTS = 12288
FEAT_DIM = 16
N_CORES = 8

IC = 512          # i-chunk width (PSUM bank)
JG = 3            # j-tiles per exp group
JT = N_POINTS // 128
NIC_PER_CORE = 3  # 1536 / 512

last_profile = {}
_program_cache = {}


def build_loop_program(T=1, N=N_POINTS, D=FEAT_DIM):
    """Per-core SPMD program with nested hardware loops. T = outer repeats."""
    key = ("loop", T, N, D)
    if key in _program_cache:
        return _program_cache[key]

    R = N // N_CORES
    n_groups = JT // JG
    assert JT % JG == 0 and R == NIC_PER_CORE * IC

    f32 = mybir.dt.float32
    f32r = mybir.dt.float32r
    EXP = mybir.ActivationFunctionType.Exp

    nc = bacc.Bacc("TRN2", target_bir_lowering=False, debug=False)

    featsT_d = nc.dram_tensor("featsT", [D, N], f32r, kind="ExternalInput")
    shardT_d = nc.dram_tensor("shardT", [D, R], f32r, kind="ExternalInput")
    bf16 = mybir.dt.bfloat16
    aug_d = nc.dram_tensor("aug", [128, JT, D + 1], bf16, kind="ExternalInput")
    out_d = nc.dram_tensor("ctx_out", [D + 1, R], f32, kind="ExternalOutput")

    with tile.TileContext(nc) as tc:
        with (
            tc.tile_pool(name="const", bufs=1) as cpool,
            tc.tile_pool(name="st", bufs=1, space="PSUM") as st_pool,
            tc.tile_pool(name="ctxp", bufs=1, space="PSUM") as ctx_pool,
            tc.tile_pool(name="e", bufs=1) as e_pool,
            tc.tile_pool(name="out", bufs=1) as out_pool,
        ):
            # featsT/shardT replicated into partition groups 0/32/64 so the
            # three matmuls of a JG-group run in distinct 32-row PE tiles
            # (tile_position row tiling) concurrently.
            feats4 = cpool.tile([128, N], f32r, name="feats4")
            shard4 = cpool.tile([128, R], f32r, name="shard4")
            for r in range(JG):
                nc.sync.dma_start(feats4[32 * r:32 * r + D, :], featsT_d[:])
                nc.sync.dma_start(shard4[32 * r:32 * r + D, :], shardT_d[:])
            aug_sb = cpool.tile([128, JT, D + 1], bf16)
            nc.sync.dma_start(aug_sb[:], aug_d[:])

            # static buffers, rotated by python index -> no pool/loop magic
            st_tiles = [st_pool.tile([128, JG * IC], f32, tag=f"st{b}", name=f"st{b}")
                        for b in range(2)]
            # fixed staging tile for the current chunk's rhs: tile_position
            # matmuls reject register-offset APs, so the dynamic chunk slice
            # is materialized here by a DVE copy (which handles them fine)
            rhs_stage = cpool.tile([128, IC], f32r, name="rhs_stage")
            E_sb = e_pool.tile([128, JT * IC], bf16, tag="E", name="E_sb")
            ctx_ps = ctx_pool.tile([D + 1, IC], f32, tag="ctx", name="ctx_ps")
            ctx_sb = out_pool.tile([D + 1, R], f32, tag="out", name="ctx_sb")

            with tc.For_i(0, T, 1, name="rep", hint_engines=(mybir.EngineType.PE,), staggered_reset=True):
                with tc.For_i(0, NIC_PER_CORE, 1, name="chunk", hint_engines=(mybir.EngineType.PE,), staggered_reset=True) as ic:
                    nc.vector.tensor_copy(rhs_stage[:], shard4[:, bass.ts(ic, IC)])
                    # Phase A: all score MMs + exp, staged into E_sb (bf16).
                    # Each jj targets a distinct PSUM bank and a distinct
                    # 32-row PE tile -> 3 concurrent score matmuls.
                    for g in range(n_groups):
                        st = st_tiles[g % 2]
                        for jj in range(JG):
                            j = g * JG + jj
                            nc.tensor.matmul(
                                st[:, jj * IC:(jj + 1) * IC],
                                feats4[32 * jj:32 * jj + D, j * 128:(j + 1) * 128],
                                rhs_stage[32 * jj:32 * jj + D, :],
                                start=True,
                                stop=True,
                                tile_position=(32 * jj, 0),
                            )
                        nc.scalar.activation(
                            E_sb[:, g * JG * IC:(g + 1) * JG * IC], st[:],
                            EXP, scale=0.25)
                    # Phase B: one uninterrupted ctx accumulation run.
                    for j in range(JT):
                        nc.tensor.matmul(
                            ctx_ps[:, :],
                            aug_sb[:, j, :],
                            E_sb[:, j * IC:(j + 1) * IC],
                            start=(j == 0),
                            stop=(j == JT - 1),
                        )
                    nc.vector.tensor_copy(ctx_sb[:, bass.ts(ic, IC)], ctx_ps[:])
                nc.sync.dma_start(out_d[:], ctx_sb[:])

    nc.compile()
    _program_cache[key] = nc
    return nc


def make_in_maps(feats, N=N_POINTS, D=FEAT_DIM):
    featsT = np.ascontiguousarray(feats.T).astype(np.float32)            # [D, N]
    aug = np.concatenate(
        [feats.astype(np.float32), np.ones((N, 1), np.float32)], axis=1
    )                                                                     # [N, D+1]
    import ml_dtypes
    aug_tiled = np.ascontiguousarray(
        aug.reshape(JT, 128, D + 1).transpose(1, 0, 2)
    ).astype(ml_dtypes.bfloat16)                                          # [128, JT, D+1]
    R = N // N_CORES
    in_maps = []
    for c in range(N_CORES):
        shardT = np.ascontiguousarray(featsT[:, c * R:(c + 1) * R])
        in_maps.append({"featsT": featsT, "shardT": shardT, "aug": aug_tiled})
    return in_maps


def run_program(nc, in_maps):
    res = None
    for attempt in range(3):
        try:
            res = bass_utils.run_bass_kernel_spmd(nc, in_maps, list(range(N_CORES)))
            break
        except Exception:
            if attempt == 2:
                raise
            time.sleep(5.0 * (attempt + 1))
    global last_profile
    last_profile = {
        "exec_time_ns": res.exec_time_ns,
        "mean_exec_time_ns": res.mean_exec_time_ns,
    }
    return res


def attention_ctx_on_device(feats, T=1):
    nc = build_loop_program(T=T)
    in_maps = make_in_maps(feats)
    res = run_program(nc, in_maps)
    ctx = np.concatenate(
        [res.results[c]["ctx_out"] for c in range(N_CORES)], axis=1
    )                                                                     # [D+1, N]
    return ctx


def _epilogue(feats, topK, ctx_aug, N, D):
    num = np.einsum("dn,nd->n", ctx_aug[:D].astype(np.float64), feats.astype(np.float64))
    Z = ctx_aug[D].astype(np.float64)
    w = 1.0 / (1.0 + np.exp(-(num / Z)))                                  # sigmoid, [N]

    weighted = feats * w[:, None].astype(np.float32)                      # [N, D]
    k = int(N * np.asarray(topK).item())
    if k >= N:
        sel = weighted
    else:
        top_idx = np.argsort(-w, kind="stable")[:k]
        sel = weighted[top_idx]
    so = (sel.T.astype(np.float32) @ sel.astype(np.float32)) / np.float32(max(k, 1))
    out = so.reshape(1, -1).astype(np.float32)
    nrm = np.linalg.norm(out, axis=-1, keepdims=True).astype(np.float32)
    return (out / nrm).astype(np.float32)


def kernel(feats, topK):
    feats = np.asarray(feats, dtype=np.float32)
    N, D = feats.shape
    ctx_aug = attention_ctx_on_device(feats, T=1)
    return _epilogue(feats, topK, ctx_aug, N, D)
